# revision 8
# baseline (speedup 1.0000x reference)
"""Single-head attention (B=16, T=2048, C=576, H=96) on 8 TRN2 NeuronCores.

Sharding: data-parallel over batch — 2 batches per core; weights replicated.

Per-core algorithm (bf16 compute, fp32 accumulation in PSUM):
  A. xT [C,T] loaded directly TRANSPOSED from DRAM via the DMA XBAR
     (dma_start_transpose, 2-byte dtype): no PE transposes, no PSUM->SBUF
     copies for x at all. x is host-padded C 576->640 so every c-tile is 128.
  B. qT,kT [96,T] = W.T @ xT (5 c-tile PSUM accumulation); v computed
     NATURAL [t128,96] (lhsT = xT tile, rhs = W) into vp [128,16,98] with
     columns 96,97 = 1.0 — the softmax denominator falls out of the output
     matmul for free.  mask is all-ones (spec fill=ones) and is ignored.
  C. scores TRANSPOSED: sT[k128, q512] = kT_chunk.T @ qT; exp on ScalarE
     straight out of PSUM (scale folded in), es written bf16 to SBUF.
     Output computed NATURAL via es-as-stationary: out[q128, 98] +=
     es[k,q].T @ v[k,98] accumulated over the 16 k-tiles.  Denominator is
     column 96; DVE reciprocal + scalar-mul normalize, store p-major
     (host unpermutes).  Softmax skips the max subtraction (scores ~N(0,1)).

Schedule: PE warmup matmuls hold the p-state ramp from t~0.5us; batch-0 xT
arrives as 20 quarter-slice XBAR DMAs so projections/scores start ~8us in;
batch-1 xT rides the DMA engine during batch-0 compute; batch-1 projections
fill PE gaps in batch-0's exp-bound C phase.  ScalarE (exp: 2*2048^2/128
lane-cycles @1.2GHz) is the roofline engine at ~66us busy.

This walrus build rejects >1 sync wait per instruction (and any wait on a
Drain), so after TileContext builds the module we hoist excess waits onto
injected same-engine NOPs — semantics identical since engines execute their
stream in order.
"""

import sys

if "/opt/trn_rl_repo" not in sys.path:
    sys.path.insert(0, "/opt/trn_rl_repo")

import numpy as np

import concourse.bass as bass
import concourse.tile as tile
from concourse import mybir
from concourse.bass_utils import run_bass_kernel_spmd

N_CORES = 8
B, T, C, H = 16, 2048, 576, 96
CP = 640  # C padded to a multiple of 128
BPC = B // N_CORES  # batches per core
SCALE = 1.0 / float(np.sqrt(H))

BF16 = mybir.dt.bfloat16
F32 = mybir.dt.float32

NCT = CP // 128  # 5 c-tiles
NT = T // 128  # 16 t/k-tiles
NQC = T // 512  # 4 query chunks
KG = 2  # k-tiles per exp group
HP = H + 2  # 98: H plus denominator column (96) plus pad for even free


def _split_excess_waits(nc, max_waits=1):
    """Hoist sync waits beyond this walrus's per-instruction limit onto
    injected NOPs that run just before, on the same engine."""
    n_split = 0
    for fn in nc.m.functions:
        for blk in fn.blocks:
            new_insts = []
            changed = False
            for inst in blk.instructions:
                si = inst.sync_info
                waits = list(si.on_wait) if si is not None else []
                cap = 0 if isinstance(inst, mybir.InstDrain) else max_waits
                if len(waits) > cap:
                    excess = waits[:-cap] if cap else waits
                    keep = waits[-cap:] if cap else []
                    for i in range(0, len(excess), max_waits):
                        chunk = excess[i : i + max_waits]
                        new_insts.append(
                            mybir.InstNoOp(
                                name=f"{inst.name}-wsplit{i}",
                                engine=inst.engine,
                                ins=[],
                                outs=[],
                                sync_info=mybir.SyncInfo(on_wait=chunk, on_update=[]),
                            )
                        )
                    inst.sync_info = mybir.SyncInfo(
                        on_wait=keep, on_update=list(si.on_update)
                    )
                    changed = True
                    n_split += 1
                new_insts.append(inst)
            if changed:
                blk.instructions = new_insts
    return n_split


def _build():
    nc = bass.Bass("TRN2", target_bir_lowering=False, debug=False)

    xp_d = nc.dram_tensor("xp", [BPC, T, CP], BF16, kind="ExternalInput")
    w_d = {
        nm: nc.dram_tensor(f"w{nm}", [128, NCT, H], BF16, kind="ExternalInput")
        for nm in "qkv"
    }
    out_d = nc.dram_tensor("out", [BPC, 128, NT, H], F32, kind="ExternalOutput")

    exp = mybir.ActivationFunctionType.Exp

    with tile.TileContext(nc) as tc:
        with (
            tc.tile_pool(name="const", bufs=1) as cpool,
            tc.tile_pool(name="xt", bufs=2) as xt_pool,
            tc.tile_pool(name="qk", bufs=2) as qk_pool,
            tc.tile_pool(name="vp", bufs=2) as vp_pool,
            tc.tile_pool(name="es", bufs=4) as es_pool,
            tc.tile_pool(name="ot", bufs=2) as ot_pool,
            tc.tile_pool(name="psm", bufs=2, space="PSUM") as psm,  # proj + warm
            tc.tile_pool(name="pss", bufs=2, space="PSUM") as pss,  # scores
            tc.tile_pool(name="pso", bufs=2, space="PSUM") as pso,  # out accum
        ):
            # ---- PE warmup: hold the p-state ramp until real work lands ----
            wm = cpool.tile([128, 512], BF16, name="wm")
            nc.vector.memset(wm[:], 0.0)
            for i in range(10):
                wps = psm.tile([128, 512], F32, tag="mm", name=f"warm{i}")
                nc.tensor.matmul(wps[:2, :], wm[:, 0:2], wm[:], start=True, stop=True)

            # ---- weight + xT DMAs --------------------------------------------
            w_sb = {}

            def emit_w(nm):
                # plain DMAs ride the ACT HWDGE queue: a plain<->transpose
                # transition on one queue serializes on DMA completion
                # (queue-level XBAR mode switch), so SP carries ONLY transposes
                wt = cpool.tile([128, NCT, H], BF16, name=f"w{nm}")
                nc.scalar.dma_start(wt[:], w_d[nm].ap())
                w_sb[nm] = wt

            state = {0: {"xt": {}}, 1: {"xt": {}}}

            # xT arrives via the DMA XBAR with a 3D out AP: the middle dim
            # folds into the partition axis, so ONE DmaTransposeAnt moves a
            # [rows, 640] block into [128, 5, rows] (xt[p, ci, r] =
            # x[r, ci*128+p]) — one DMA issue per quarter (HWDGE holds the
            # sequencer ~650ns per DMA, so issue count matters).
            def emit_xbar_quarter(b, q):
                t = xt_pool.tile([128, NCT, 512], BF16, tag=f"xtq{q}", name=f"xtq{q}_{b}")
                state[b]["xt"][q] = t
                nc.sync.dma_start(
                    t[:], xp_d.ap()[b, q * 512 : (q + 1) * 512, :], transpose=True
                )

            def emit_xbar_full(b):
                t = xt_pool.tile([128, NCT, T], BF16, tag="xtf", name=f"xtf_{b}")
                for q in range(4):
                    state[b]["xt"][q] = t
                state[b]["xt_full"] = True
                nc.sync.dma_start(t[:], xp_d.ap()[b], transpose=True)

            def xtv(b, q, ci, lo, hi):
                """xT slice [128, hi-lo] of quarter q, c-tile ci (cols within
                the quarter)."""
                st = state[b]
                if st.get("xt_full"):
                    return st["xt"][q][:, ci, q * 512 + lo : q * 512 + hi]
                return st["xt"][q][:, ci, lo:hi]

            # ---- projection units -------------------------------------------
            def u_qk(b, nm, ch):
                def go():
                    st = state[b]
                    key = f"t{nm}"
                    if key not in st:
                        st[key] = qk_pool.tile(
                            [H, T], BF16, tag=key, name=f"{key}_{b}"
                        )
                    pp = psm.tile([128, 512], F32, tag="mm", name=f"p{nm}{ch}_{b}")
                    for ci in range(NCT):
                        nc.tensor.matmul(
                            pp[:H, :],
                            w_sb[nm][:, ci, :],
                            xtv(b, ch, ci, 0, 512),
                            start=(ci == 0),
                            stop=(ci == NCT - 1),
                        )
                    nc.vector.tensor_copy(
                        st[key][:, ch * 512 : (ch + 1) * 512], pp[:H, :]
                    )

                return go

            def u_v(b, g):
                def go():
                    st = state[b]
                    if "vp" not in st:
                        st["vp"] = vp_pool.tile(
                            [128, NT, HP], BF16, tag="vp", name=f"vp_{b}"
                        )
                        nc.vector.memset(st["vp"][:, :, H:HP], 1.0)
                    # one PSUM zero-region (2KB bank) holds 4 accumulation
                    # islands: start zeroes the whole bank, so only the first
                    # matmul starts and only the last stops
                    pv = psm.tile([128, 512], F32, tag="mm", name=f"pv{g}_{b}")
                    for j in range(4):
                        tt = g * 4 + j
                        for ci in range(NCT):
                            nc.tensor.matmul(
                                pv[:, j * H : (j + 1) * H],
                                xtv(b, g, ci, j * 128, (j + 1) * 128),
                                w_sb["v"][:, ci, :],
                                start=(ci == 0 and j == 0),
                                stop=(ci == NCT - 1 and j == 3),
                                skip_group_check=(ci == 0 and j > 0),
                            )
                    nc.vector.tensor_copy(
                        st["vp"][:, g * 4 : (g + 1) * 4, :H],
                        pv[:, : 4 * H].rearrange("p (j h) -> p j h", h=H),
                    )

                return go

            # ---- attention C phase ------------------------------------------
            def c_pass(b, qc, ktg_lo, ktg_hi, fill):
                """ktg groups [ktg_lo, ktg_hi) of one 512-query chunk:
                (2 scores, 1 exp, 8 output-accumulate matmuls) per group; on
                the final group, normalize + store.  fill: iterator of
                emit-closures interleaved once per ktg."""
                st = state[b]
                tq, tk = st["tq"], st["tk"]
                okey = f"ops{qc}"
                if okey not in st:
                    st[okey] = pso.tile([128, NQC, HP], F32, tag="o", name=f"{okey}_{b}")
                ops = st[okey]
                for ktg in range(ktg_lo, ktg_hi):
                    sps = pss.tile([128, KG, 512], F32, tag="s", name=f"sps_{b}")
                    for j in range(KG):
                        kt = ktg * KG + j
                        nc.tensor.matmul(
                            sps[:, j, :],
                            tk[:, kt * 128 : (kt + 1) * 128],
                            tq[:, qc * 512 : (qc + 1) * 512],
                            start=True,
                            stop=True,
                        )
                    es = es_pool.tile([128, KG, 512], BF16, tag="es", name=f"es_{b}")
                    nc.scalar.activation(es[:], sps[:], exp, scale=SCALE)
                    if fill is not None:
                        u = next(fill, None)
                        if u is not None:
                            u()
                    for qt in range(4):
                        for j in range(KG):
                            kt = ktg * KG + j
                            nc.tensor.matmul(
                                ops[:, qt, :],
                                es[:, j, qt * 128 : (qt + 1) * 128],
                                st["vp"][:, kt, :],
                                start=(kt == 0 and qt == 0),
                                stop=(kt == NT - 1 and qt == 3),
                                skip_group_check=(kt == 0 and qt > 0),
                            )
                if ktg_hi < NT // KG:
                    return
                del st[okey]
                rec = ot_pool.tile([128, NQC], F32, tag="rec", name=f"rec{qc}_{b}")
                nc.vector.reciprocal(
                    rec[:], ops[:, :, H : H + 1].rearrange("p a o -> p (a o)")
                )
                ot = ot_pool.tile([128, NQC, H], F32, tag="ot", name=f"ot{qc}_{b}")
                for qt in range(4):
                    nc.vector.tensor_scalar_mul(
                        ot[:, qt, :], ops[:, qt, :H], rec[:, qt : qt + 1]
                    )
                nc.scalar.dma_start(out_d.ap()[b, :, qc * 4 : (qc + 1) * 4, :], ot[:])

            # ---- emission schedule ------------------------------------------
            emit_w("k")
            emit_w("q")
            emit_w("v")
            for q in range(4):
                emit_xbar_quarter(0, q)
            emit_xbar_full(1)

            # batch 0 projections, progressively interleaved with the C phase.
            # qc0/qc1 run in two half-key passes so exp never waits on the
            # tail of the x-transpose DMA stream.
            u_qk(0, "k", 0)()
            u_qk(0, "q", 0)()
            f1 = iter([u_v(0, 0), u_qk(0, "k", 1), u_v(0, 1), u_qk(0, "q", 1)])
            c_pass(0, 0, 0, 4, f1)
            f2 = iter([u_qk(0, "k", 2), u_v(0, 2), u_qk(0, "q", 2)])
            c_pass(0, 1, 0, 4, f2)
            f3 = iter([u_qk(0, "k", 3), u_v(0, 3), u_qk(0, "q", 3)])
            c_pass(0, 0, 4, 8, f3)
            b1_units = iter(
                [
                    u_qk(1, "k", 0),
                    u_qk(1, "q", 0),
                    u_v(1, 0),
                    u_qk(1, "k", 1),
                    u_v(1, 1),
                    u_qk(1, "k", 2),
                    u_v(1, 2),
                    u_qk(1, "k", 3),
                    u_v(1, 3),
                    u_qk(1, "q", 1),
                    u_qk(1, "q", 2),
                    u_qk(1, "q", 3),
                ]
            )
            c_pass(0, 1, 4, 8, b1_units)
            c_pass(0, 2, 0, 8, b1_units)
            c_pass(0, 3, 0, 8, b1_units)
            c_pass(1, 0, 0, 8, b1_units)
            c_pass(1, 1, 0, 8, None)
            c_pass(1, 2, 0, 8, None)
            c_pass(1, 3, 0, 8, None)

    _split_excess_waits(nc)
    return nc


_prog = None


def _get_prog():
    global _prog
    if _prog is None:
        _prog = _build()
    return _prog


def _to_bf16(a):
    import ml_dtypes

    return np.asarray(a, dtype=ml_dtypes.bfloat16)


def kernel(x, mask, Wk, Wq, Wv, **_ignored):
    x = np.ascontiguousarray(x, dtype=np.float32)
    xp = np.zeros((B, T, CP), dtype=np.float32)
    xp[:, :, :C] = x
    xp = _to_bf16(xp)

    wt = {}
    for nm, W in (("q", Wq), ("k", Wk), ("v", Wv)):
        wp = np.zeros((CP, H), dtype=np.float32)
        wp[:C] = np.asarray(W, dtype=np.float32)
        wt[nm] = np.ascontiguousarray(
            _to_bf16(wp.reshape(NCT, 128, H).transpose(1, 0, 2))
        )

    nc = _get_prog()
    in_maps = [
        {
            "xp": xp[i * BPC : (i + 1) * BPC],
            "wq": wt["q"],
            "wk": wt["k"],
            "wv": wt["v"],
        }
        for i in range(N_CORES)
    ]
    res = run_bass_kernel_spmd(nc, in_maps, core_ids=list(range(N_CORES)))
    # out is p-major [BPC, 128, NT, H]: unpermute to [BPC, T, H]
    outs = []
    for i in range(N_CORES):
        o = res.results[i]["out"]
        outs.append(o.transpose(0, 2, 1, 3).reshape(BPC, T, H))
    return np.concatenate(outs, axis=0)


if __name__ == "__main__":
    rng = np.random.default_rng(0)
    x = rng.standard_normal((B, T, C), dtype=np.float32)
    mask = np.ones((B, T), dtype=bool)
    s = 1.0 / np.sqrt(C)
    Wk = (rng.standard_normal((C, H)) * s).astype(np.float32)
    Wq = (rng.standard_normal((C, H)) * s).astype(np.float32)
    Wv = (rng.standard_normal((C, H)) * s).astype(np.float32)
    out = kernel(x, mask=mask, Wk=Wk, Wq=Wq, Wv=Wv)
    print("out", out.shape, out.dtype, float(np.abs(out).max()))


# revision 15
# speedup vs baseline: 1.1155x; 1.1155x over previous
"""Single-head attention (B=16, T=2048, C=576, H=96) on 8 TRN2 NeuronCores.

Sharding: data-parallel over batch — 2 batches per core; weights replicated.

Per-core algorithm (bf16 compute, fp32 accumulation in PSUM):
  A. xT [C,T] loaded directly TRANSPOSED from DRAM via the DMA XBAR
     (dma_start_transpose, 2-byte dtype): no PE transposes, no PSUM->SBUF
     copies for x at all. x is host-padded C 576->640 so every c-tile is 128.
  B. qT,kT [96,T] = W.T @ xT (5 c-tile PSUM accumulation); v computed
     NATURAL [t128,96] (lhsT = xT tile, rhs = W) into vp [128,16,98] with
     columns 96,97 = 1.0 — the softmax denominator falls out of the output
     matmul for free.  mask is all-ones (spec fill=ones) and is ignored.
  C. scores TRANSPOSED: sT[k128, q512] = kT_chunk.T @ qT; exp on ScalarE
     straight out of PSUM (scale folded in), es written bf16 to SBUF.
     Output computed NATURAL via es-as-stationary: out[q128, 98] +=
     es[k,q].T @ v[k,98] accumulated over the 16 k-tiles.  Denominator is
     column 96; DVE reciprocal + scalar-mul normalize, store p-major
     (host unpermutes).  Softmax skips the max subtraction (scores ~N(0,1)).

Schedule: PE warmup matmuls hold the p-state ramp from t~0.5us; batch-0 xT
arrives as 20 quarter-slice XBAR DMAs so projections/scores start ~8us in;
batch-1 xT rides the DMA engine during batch-0 compute; batch-1 projections
fill PE gaps in batch-0's exp-bound C phase.  ScalarE (exp: 2*2048^2/128
lane-cycles @1.2GHz) is the roofline engine at ~66us busy.

This walrus build rejects >1 sync wait per instruction (and any wait on a
Drain), so after TileContext builds the module we hoist excess waits onto
injected same-engine NOPs — semantics identical since engines execute their
stream in order.
"""

import sys

if "/opt/trn_rl_repo" not in sys.path:
    sys.path.insert(0, "/opt/trn_rl_repo")

import numpy as np

import concourse.bass as bass
import concourse.tile as tile
from concourse import mybir
from concourse.bass_utils import run_bass_kernel_spmd

N_CORES = 8
B, T, C, H = 16, 2048, 576, 96
CP = 640  # C padded to a multiple of 128
BPC = B // N_CORES  # batches per core
SCALE = 1.0 / float(np.sqrt(H))

BF16 = mybir.dt.bfloat16
FP8 = mybir.dt.float8e4
F32 = mybir.dt.float32

NCT = CP // 128  # 5 c-tiles
NT = T // 128  # 16 t/k-tiles
NQC = T // 512  # 4 query chunks
KG = 2  # k-tiles per exp group
HP = H + 2  # 98: H plus denominator column (96) plus pad for even free


def _split_excess_waits(nc, max_waits=1):
    """Hoist sync waits beyond this walrus's per-instruction limit onto
    injected NOPs that run just before, on the same engine."""
    n_split = 0
    for fn in nc.m.functions:
        for blk in fn.blocks:
            new_insts = []
            changed = False
            for inst in blk.instructions:
                si = inst.sync_info
                waits = list(si.on_wait) if si is not None else []
                cap = 0 if isinstance(inst, mybir.InstDrain) else max_waits
                if len(waits) > cap:
                    excess = waits[:-cap] if cap else waits
                    keep = waits[-cap:] if cap else []
                    for i in range(0, len(excess), max_waits):
                        chunk = excess[i : i + max_waits]
                        new_insts.append(
                            mybir.InstNoOp(
                                name=f"{inst.name}-wsplit{i}",
                                engine=inst.engine,
                                ins=[],
                                outs=[],
                                sync_info=mybir.SyncInfo(on_wait=chunk, on_update=[]),
                            )
                        )
                    inst.sync_info = mybir.SyncInfo(
                        on_wait=keep, on_update=list(si.on_update)
                    )
                    changed = True
                    n_split += 1
                new_insts.append(inst)
            if changed:
                blk.instructions = new_insts
    return n_split


def _build():
    nc = bass.Bass("TRN2", target_bir_lowering=False, debug=False)

    xp_d = nc.dram_tensor("xp", [BPC, T, CP], BF16, kind="ExternalInput")
    w_d = nc.dram_tensor("wall", [3 * H, CP], BF16, kind="ExternalInput")
    out_d = nc.dram_tensor("out", [BPC, 128, NT, H], F32, kind="ExternalOutput")

    exp = mybir.ActivationFunctionType.Exp

    with tile.TileContext(nc) as tc:
        with (
            tc.tile_pool(name="const", bufs=1) as cpool,
            tc.tile_pool(name="xt", bufs=2) as xt_pool,
            tc.tile_pool(name="qk", bufs=2) as qk_pool,
            tc.tile_pool(name="vp", bufs=2) as vp_pool,
            tc.tile_pool(name="es", bufs=4) as es_pool,
            tc.tile_pool(name="ot", bufs=2) as ot_pool,
            tc.tile_pool(name="psm", bufs=2, space="PSUM") as psm,  # proj + warm
            tc.tile_pool(name="pss", bufs=2, space="PSUM") as pss,  # scores
            tc.tile_pool(name="pso", bufs=2, space="PSUM") as pso,  # out accum
        ):
            # ---- PE warmup: hold the p-state ramp until real work lands ----
            wm = cpool.tile([128, 512], BF16, name="wm")
            nc.vector.memset(wm[:], 0.0)
            for i in range(10):
                wps = psm.tile([128, 512], F32, tag="mm", name=f"warm{i}")
                nc.tensor.matmul(wps[:2, :], wm[:, 0:2], wm[:], start=True, stop=True)

            # ---- weight + xT DMAs --------------------------------------------
            # ALL DMAs ride one queue (SP) ordered transposes-first: any
            # plain<->transpose transition serializes on DMA completion
            # globally (XBAR mode switch), so weights load via the XBAR too:
            # host packs W.T [3H, CP]; transpose yields [128, ci, nm*H+h] =
            # W_nm[ci*128+p, h].
            wall = cpool.tile([128, NCT, 3 * H], BF16, name="wall")
            nc.sync.dma_start(wall[:], w_d.ap(), transpose=True)
            NMI = {"q": 0, "k": 1, "v": 2}

            def w_sb(nm, ci):
                i = NMI[nm]
                return wall[:, ci, i * H : (i + 1) * H]

            state = {0: {"xt": {}}, 1: {"xt": {}}}

            # xT arrives via the DMA XBAR with a 3D out AP: the middle dim
            # folds into the partition axis, so ONE DmaTransposeAnt moves a
            # [rows, 640] block into [128, 5, rows] (xt[p, ci, r] =
            # x[r, ci*128+p]) — one DMA issue per quarter (HWDGE holds the
            # sequencer ~650ns per DMA, so issue count matters).
            def emit_xbar_quarter(b, q):
                t = xt_pool.tile([128, NCT, 512], BF16, tag=f"xtq{q}", name=f"xtq{q}_{b}")
                state[b]["xt"][q] = t
                nc.sync.dma_start(
                    t[:], xp_d.ap()[b, q * 512 : (q + 1) * 512, :], transpose=True
                )

            def emit_xbar_full(b):
                t = xt_pool.tile([128, NCT, T], BF16, tag="xtf", name=f"xtf_{b}")
                for q in range(4):
                    state[b]["xt"][q] = t
                state[b]["xt_full"] = True
                nc.sync.dma_start(t[:], xp_d.ap()[b], transpose=True)

            def xtv(b, q, ci, lo, hi):
                """xT slice [128, hi-lo] of quarter q, c-tile ci (cols within
                the quarter)."""
                st = state[b]
                if st.get("xt_full"):
                    return st["xt"][q][:, ci, q * 512 + lo : q * 512 + hi]
                return st["xt"][q][:, ci, lo:hi]

            # ---- projection units -------------------------------------------
            def u_qk(b, nm, ch):
                def go():
                    st = state[b]
                    key = f"t{nm}"
                    if key not in st:
                        st[key] = qk_pool.tile(
                            [H, T], BF16, tag=key, name=f"{key}_{b}"
                        )
                    pp = psm.tile([128, 512], F32, tag="mm", name=f"p{nm}{ch}_{b}")
                    for ci in range(NCT):
                        nc.tensor.matmul(
                            pp[:H, :],
                            w_sb(nm, ci),
                            xtv(b, ch, ci, 0, 512),
                            start=(ci == 0),
                            stop=(ci == NCT - 1),
                        )
                    nc.vector.tensor_copy(
                        st[key][:, ch * 512 : (ch + 1) * 512], pp[:H, :]
                    )

                return go

            def u_v(b, g):
                def go():
                    st = state[b]
                    if "vp" not in st:
                        st["vp"] = vp_pool.tile(
                            [128, NT, HP], BF16, tag="vp", name=f"vp_{b}"
                        )
                        nc.vector.memset(st["vp"][:, :, H:HP], 1.0)
                    # one PSUM zero-region (2KB bank) holds 4 accumulation
                    # islands: start zeroes the whole bank, so only the first
                    # matmul starts and only the last stops
                    pv = psm.tile([128, 512], F32, tag="mm", name=f"pv{g}_{b}")
                    for j in range(4):
                        tt = g * 4 + j
                        for ci in range(NCT):
                            nc.tensor.matmul(
                                pv[:, j * H : (j + 1) * H],
                                xtv(b, g, ci, j * 128, (j + 1) * 128),
                                w_sb("v", ci),
                                start=(ci == 0 and j == 0),
                                stop=(ci == NCT - 1 and j == 3),
                                skip_group_check=(ci == 0 and j > 0),
                            )
                    nc.vector.tensor_copy(
                        st["vp"][:, g * 4 : (g + 1) * 4, :H],
                        pv[:, : 4 * H].rearrange("p (j h) -> p j h", h=H),
                    )

                return go

            # ---- attention C phase ------------------------------------------
            def c_pass(b, qc, ktg_lo, ktg_hi, fill):
                """ktg groups [ktg_lo, ktg_hi) of one 512-query chunk:
                (2 scores, 1 exp, 8 output-accumulate matmuls) per group; on
                the final group, normalize + store.  fill: iterator of
                emit-closures interleaved once per ktg."""
                st = state[b]
                tq, tk = st["tq"], st["tk"]
                okey = f"ops{qc}"
                if okey not in st:
                    st[okey] = pso.tile([128, NQC, HP], F32, tag="o", name=f"{okey}_{b}")
                ops = st[okey]
                for ktg in range(ktg_lo, ktg_hi):
                    sps = pss.tile([128, KG, 512], F32, tag="s", name=f"sps_{b}")
                    for j in range(KG):
                        kt = ktg * KG + j
                        nc.tensor.matmul(
                            sps[:, j, :],
                            tk[:, kt * 128 : (kt + 1) * 128],
                            tq[:, qc * 512 : (qc + 1) * 512],
                            start=True,
                            stop=True,
                        )
                    es = es_pool.tile([128, KG, 512], BF16, tag="es", name=f"es_{b}")
                    nc.scalar.activation(es[:], sps[:], exp, scale=SCALE)
                    if fill is not None:
                        u = next(fill, None)
                        if u is not None:
                            u()
                    for qt in range(4):
                        for j in range(KG):
                            kt = ktg * KG + j
                            nc.tensor.matmul(
                                ops[:, qt, :],
                                es[:, j, qt * 128 : (qt + 1) * 128],
                                st["vp"][:, kt, :],
                                start=(kt == 0 and qt == 0),
                                stop=(kt == NT - 1 and qt == 3),
                                skip_group_check=(kt == 0 and qt > 0),
                            )
                if ktg_hi < NT // KG:
                    return
                del st[okey]
                rec = ot_pool.tile([128, NQC], F32, tag="rec", name=f"rec{qc}_{b}")
                nc.vector.reciprocal(
                    rec[:], ops[:, :, H : H + 1].rearrange("p a o -> p (a o)")
                )
                ot = ot_pool.tile([128, NQC, H], F32, tag="ot", name=f"ot{qc}_{b}")
                for qt in range(4):
                    nc.vector.tensor_scalar_mul(
                        ot[:, qt, :], ops[:, qt, :H], rec[:, qt : qt + 1]
                    )
                nc.sync.dma_start(out_d.ap()[b, :, qc * 4 : (qc + 1) * 4, :], ot[:])

            # ---- emission schedule ------------------------------------------
            for q in range(4):
                emit_xbar_quarter(0, q)
            emit_xbar_full(1)

            # batch 0 projections, progressively interleaved with the C phase.
            # qc0/qc1 run in two half-key passes so exp never waits on the
            # tail of the x-transpose DMA stream.
            u_qk(0, "k", 0)()
            u_qk(0, "q", 0)()
            f0 = iter(
                [
                    u_v(0, 0),
                    u_qk(0, "k", 1),
                    u_v(0, 1),
                    u_qk(0, "k", 2),
                    u_v(0, 2),
                    u_qk(0, "k", 3),
                    u_v(0, 3),
                    u_qk(0, "q", 1),
                ]
            )
            c_pass(0, 0, 0, 8, f0)
            f1 = iter(
                [
                    u_qk(0, "q", 2),
                    u_qk(0, "q", 3),
                    u_qk(1, "k", 0),
                    u_qk(1, "q", 0),
                    u_v(1, 0),
                    u_qk(1, "k", 1),
                    u_v(1, 1),
                    u_qk(1, "k", 2),
                ]
            )
            c_pass(0, 1, 0, 8, f1)
            f2 = iter(
                [
                    u_v(1, 2),
                    u_qk(1, "k", 3),
                    u_v(1, 3),
                    u_qk(1, "q", 1),
                    u_qk(1, "q", 2),
                    u_qk(1, "q", 3),
                ]
            )
            c_pass(0, 2, 0, 8, f2)
            c_pass(0, 3, 0, 8, f2)
            c_pass(1, 0, 0, 8, None)
            c_pass(1, 1, 0, 8, None)
            c_pass(1, 2, 0, 8, None)
            c_pass(1, 3, 0, 8, None)

    _split_excess_waits(nc)
    return nc


_prog = None


def _get_prog():
    global _prog
    if _prog is None:
        _prog = _build()
    return _prog


def _to_bf16(a):
    import ml_dtypes

    return np.asarray(a, dtype=ml_dtypes.bfloat16)


def kernel(x, mask, Wk, Wq, Wv, **_ignored):
    x = np.ascontiguousarray(x, dtype=np.float32)
    xp = np.zeros((B, T, CP), dtype=np.float32)
    xp[:, :, :C] = x
    xp = _to_bf16(xp)

    wall = np.zeros((3 * H, CP), dtype=np.float32)
    for i, W in enumerate((Wq, Wk, Wv)):
        wall[i * H : (i + 1) * H, :C] = np.asarray(W, dtype=np.float32).T
    wall = np.ascontiguousarray(_to_bf16(wall))

    nc = _get_prog()
    in_maps = [
        {"xp": xp[i * BPC : (i + 1) * BPC], "wall": wall}
        for i in range(N_CORES)
    ]
    res = run_bass_kernel_spmd(nc, in_maps, core_ids=list(range(N_CORES)))
    # out is p-major [BPC, 128, NT, H]: unpermute to [BPC, T, H]
    outs = []
    for i in range(N_CORES):
        o = res.results[i]["out"]
        outs.append(o.transpose(0, 2, 1, 3).reshape(BPC, T, H))
    return np.concatenate(outs, axis=0)


if __name__ == "__main__":
    rng = np.random.default_rng(0)
    x = rng.standard_normal((B, T, C), dtype=np.float32)
    mask = np.ones((B, T), dtype=bool)
    s = 1.0 / np.sqrt(C)
    Wk = (rng.standard_normal((C, H)) * s).astype(np.float32)
    Wq = (rng.standard_normal((C, H)) * s).astype(np.float32)
    Wv = (rng.standard_normal((C, H)) * s).astype(np.float32)
    out = kernel(x, mask=mask, Wk=Wk, Wq=Wq, Wv=Wv)
    print("out", out.shape, out.dtype, float(np.abs(out).max()))


# revision 16
# speedup vs baseline: 1.2252x; 1.0984x over previous
"""Single-head attention (B=16, T=2048, C=576, H=96) on 8 TRN2 NeuronCores.

Sharding: data-parallel over batch — 2 batches per core; weights replicated.

Per-core algorithm (bf16 compute, fp32 accumulation in PSUM):
  A. xT [C,T] loaded directly TRANSPOSED from DRAM via the DMA XBAR
     (dma_start_transpose, 2-byte dtype): no PE transposes, no PSUM->SBUF
     copies for x at all. x is host-padded C 576->640 so every c-tile is 128.
  B. qT,kT [96,T] = W.T @ xT (5 c-tile PSUM accumulation); v computed
     NATURAL [t128,96] (lhsT = xT tile, rhs = W) into vp [128,16,98] with
     columns 96,97 = 1.0 — the softmax denominator falls out of the output
     matmul for free.  mask is all-ones (spec fill=ones) and is ignored.
  C. scores TRANSPOSED: sT[k128, q512] = kT_chunk.T @ qT; exp on ScalarE
     straight out of PSUM (scale folded in), es written bf16 to SBUF.
     Output computed NATURAL via es-as-stationary: out[q128, 98] +=
     es[k,q].T @ v[k,98] accumulated over the 16 k-tiles.  Denominator is
     column 96; DVE reciprocal + scalar-mul normalize, store p-major
     (host unpermutes).  Softmax skips the max subtraction (scores ~N(0,1)).

Schedule: PE warmup matmuls hold the p-state ramp from t~0.5us; batch-0 xT
arrives as 20 quarter-slice XBAR DMAs so projections/scores start ~8us in;
batch-1 xT rides the DMA engine during batch-0 compute; batch-1 projections
fill PE gaps in batch-0's exp-bound C phase.  ScalarE (exp: 2*2048^2/128
lane-cycles @1.2GHz) is the roofline engine at ~66us busy.

This walrus build rejects >1 sync wait per instruction (and any wait on a
Drain), so after TileContext builds the module we hoist excess waits onto
injected same-engine NOPs — semantics identical since engines execute their
stream in order.
"""

import sys

if "/opt/trn_rl_repo" not in sys.path:
    sys.path.insert(0, "/opt/trn_rl_repo")

import numpy as np

import concourse.bass as bass
import concourse.tile as tile
from concourse import mybir
from concourse.bass_utils import run_bass_kernel_spmd

N_CORES = 8
B, T, C, H = 16, 2048, 576, 96
CP = 640  # C padded to a multiple of 128
BPC = B // N_CORES  # batches per core
SCALE = 1.0 / float(np.sqrt(H))

BF16 = mybir.dt.bfloat16
FP8 = mybir.dt.float8e4
F32 = mybir.dt.float32

NCT = CP // 128  # 5 c-tiles
NT = T // 128  # 16 t/k-tiles
NQC = T // 512  # 4 query chunks
KG = 2  # k-tiles per exp group
HP = H + 2  # 98: H plus denominator column (96) plus pad for even free


def _split_excess_waits(nc, max_waits=1):
    """Hoist sync waits beyond this walrus's per-instruction limit onto
    injected NOPs that run just before, on the same engine."""
    n_split = 0
    for fn in nc.m.functions:
        for blk in fn.blocks:
            new_insts = []
            changed = False
            for inst in blk.instructions:
                si = inst.sync_info
                waits = list(si.on_wait) if si is not None else []
                cap = 0 if isinstance(inst, mybir.InstDrain) else max_waits
                if len(waits) > cap:
                    excess = waits[:-cap] if cap else waits
                    keep = waits[-cap:] if cap else []
                    for i in range(0, len(excess), max_waits):
                        chunk = excess[i : i + max_waits]
                        new_insts.append(
                            mybir.InstNoOp(
                                name=f"{inst.name}-wsplit{i}",
                                engine=inst.engine,
                                ins=[],
                                outs=[],
                                sync_info=mybir.SyncInfo(on_wait=chunk, on_update=[]),
                            )
                        )
                    inst.sync_info = mybir.SyncInfo(
                        on_wait=keep, on_update=list(si.on_update)
                    )
                    changed = True
                    n_split += 1
                new_insts.append(inst)
            if changed:
                blk.instructions = new_insts
    return n_split


def _build():
    nc = bass.Bass("TRN2", target_bir_lowering=False, debug=False)

    xp_d = nc.dram_tensor("xp", [BPC, T, CP], BF16, kind="ExternalInput")
    w_d = nc.dram_tensor("wall", [3 * H, CP], BF16, kind="ExternalInput")
    out_d = nc.dram_tensor("out", [BPC, 128, NT, H], F32, kind="ExternalOutput")

    exp = mybir.ActivationFunctionType.Exp

    with tile.TileContext(nc) as tc:
        with (
            tc.tile_pool(name="const", bufs=1) as cpool,
            tc.tile_pool(name="xt", bufs=2) as xt_pool,
            tc.tile_pool(name="qk", bufs=2) as qk_pool,
            tc.tile_pool(name="vp", bufs=2) as vp_pool,
            tc.tile_pool(name="es", bufs=4) as es_pool,
            tc.tile_pool(name="ot", bufs=2) as ot_pool,
            tc.tile_pool(name="psm", bufs=2, space="PSUM") as psm,  # proj + warm
            tc.tile_pool(name="pss", bufs=2, space="PSUM") as pss,  # scores
            tc.tile_pool(name="pso", bufs=2, space="PSUM") as pso,  # out accum
        ):
            # ---- PE warmup: hold the p-state ramp until real work lands ----
            wm = cpool.tile([128, 512], BF16, name="wm")
            nc.vector.memset(wm[:], 0.0)
            for i in range(10):
                wps = psm.tile([128, 512], F32, tag="mm", name=f"warm{i}")
                nc.tensor.matmul(wps[:2, :], wm[:, 0:2], wm[:], start=True, stop=True)

            # ---- weight + xT DMAs --------------------------------------------
            # ALL DMAs ride one queue (SP) ordered transposes-first: any
            # plain<->transpose transition serializes on DMA completion
            # globally (XBAR mode switch), so weights load via the XBAR too:
            # host packs W.T [3H, CP]; transpose yields [128, ci, nm*H+h] =
            # W_nm[ci*128+p, h].
            wall = cpool.tile([128, NCT, 3 * H], BF16, name="wall")
            nc.sync.dma_start(wall[:], w_d.ap(), transpose=True)
            NMI = {"q": 0, "k": 1, "v": 2}

            def w_sb(nm, ci):
                i = NMI[nm]
                return wall[:, ci, i * H : (i + 1) * H]

            state = {0: {"xt": {}}, 1: {"xt": {}}}

            # xT arrives via the DMA XBAR with a 3D out AP: the middle dim
            # folds into the partition axis, so ONE DmaTransposeAnt moves a
            # [rows, 640] block into [128, 5, rows] (xt[p, ci, r] =
            # x[r, ci*128+p]) — one DMA issue per quarter (HWDGE holds the
            # sequencer ~650ns per DMA, so issue count matters).
            def emit_xbar_quarter(b, q):
                t = xt_pool.tile([128, NCT, 512], BF16, tag=f"xtq{q}", name=f"xtq{q}_{b}")
                state[b]["xt"][q] = t
                nc.sync.dma_start(
                    t[:], xp_d.ap()[b, q * 512 : (q + 1) * 512, :], transpose=True
                )

            def emit_xbar_full(b):
                t = xt_pool.tile([128, NCT, T], BF16, tag="xtf", name=f"xtf_{b}")
                for q in range(4):
                    state[b]["xt"][q] = t
                state[b]["xt_full"] = True
                nc.sync.dma_start(t[:], xp_d.ap()[b], transpose=True)

            def xtv(b, q, ci, lo, hi):
                """xT slice [128, hi-lo] of quarter q, c-tile ci (cols within
                the quarter)."""
                st = state[b]
                if st.get("xt_full"):
                    return st["xt"][q][:, ci, q * 512 + lo : q * 512 + hi]
                return st["xt"][q][:, ci, lo:hi]

            # ---- projection units -------------------------------------------
            # each unit is split into sub-~500ns pieces so a C-phase ktg slot
            # (exp budget 1038ns, PE base ~550ns) never overruns ScalarE
            def u_qk(b, nm, ch):
                box = {}

                def part(ci_lo, ci_hi, copy):
                    def go():
                        st = state[b]
                        key = f"t{nm}"
                        if key not in st:
                            st[key] = qk_pool.tile(
                                [H, T], BF16, tag=key, name=f"{key}_{b}"
                            )
                        if "pp" not in box:
                            box["pp"] = psm.tile(
                                [128, 512], F32, tag="mm", name=f"p{nm}{ch}_{b}"
                            )
                        pp = box["pp"]
                        for ci in range(ci_lo, ci_hi):
                            nc.tensor.matmul(
                                pp[:H, :],
                                w_sb(nm, ci),
                                xtv(b, ch, ci, 0, 512),
                                start=(ci == 0),
                                stop=(ci == NCT - 1),
                            )
                        if copy:
                            nc.vector.tensor_copy(
                                st[key][:, ch * 512 : (ch + 1) * 512], pp[:H, :]
                            )

                    return go

                return [part(0, 2, False), part(2, 4, False), part(4, NCT, True)]

            def u_v(b, g):
                box = {}

                def part(j_lo, j_hi, copy):
                    def go():
                        st = state[b]
                        if "vp" not in st:
                            st["vp"] = vp_pool.tile(
                                [128, NT, HP], BF16, tag="vp", name=f"vp_{b}"
                            )
                            nc.vector.memset(st["vp"][:, :, H:HP], 1.0)
                        if "pv" not in box:
                            box["pv"] = psm.tile(
                                [128, 512], F32, tag="mm", name=f"pv{g}_{b}"
                            )
                        pv = box["pv"]
                        # 4 accumulation islands share one zero region: only
                        # the first matmul starts, only the last stops
                        for j in range(j_lo, j_hi):
                            tt = g * 4 + j
                            for ci in range(NCT):
                                nc.tensor.matmul(
                                    pv[:, j * H : (j + 1) * H],
                                    xtv(b, g, ci, j * 128, (j + 1) * 128),
                                    w_sb("v", ci),
                                    start=(ci == 0 and j == 0),
                                    stop=(ci == NCT - 1 and j == 3),
                                    skip_group_check=(ci == 0 and j > 0),
                                )
                        if copy:
                            nc.vector.tensor_copy(
                                st["vp"][:, g * 4 : (g + 1) * 4, :H],
                                pv[:, : 4 * H].rearrange("p (j h) -> p j h", h=H),
                            )

                    return go

                return [part(0, 2, False), part(2, 4, True)]

            # ---- attention C phase ------------------------------------------
            def c_pass(b, qc, ktg_lo, ktg_hi, fill):
                """ktg groups [ktg_lo, ktg_hi) of one 512-query chunk:
                (2 scores, 1 exp, 8 output-accumulate matmuls) per group; on
                the final group, normalize + store.  fill: iterator of
                emit-closures interleaved once per ktg."""
                st = state[b]
                tq, tk = st["tq"], st["tk"]
                okey = f"ops{qc}"
                if okey not in st:
                    st[okey] = pso.tile([128, NQC, HP], F32, tag="o", name=f"{okey}_{b}")
                ops = st[okey]
                for ktg in range(ktg_lo, ktg_hi):
                    sps = pss.tile([128, KG, 512], F32, tag="s", name=f"sps_{b}")
                    for j in range(KG):
                        kt = ktg * KG + j
                        nc.tensor.matmul(
                            sps[:, j, :],
                            tk[:, kt * 128 : (kt + 1) * 128],
                            tq[:, qc * 512 : (qc + 1) * 512],
                            start=True,
                            stop=True,
                        )
                    es = es_pool.tile([128, KG, 512], BF16, tag="es", name=f"es_{b}")
                    nc.scalar.activation(es[:], sps[:], exp, scale=SCALE)
                    if fill is not None:
                        u = next(fill, None)
                        if u is not None:
                            u()
                    for qt in range(4):
                        for j in range(KG):
                            kt = ktg * KG + j
                            nc.tensor.matmul(
                                ops[:, qt, :],
                                es[:, j, qt * 128 : (qt + 1) * 128],
                                st["vp"][:, kt, :],
                                start=(kt == 0 and qt == 0),
                                stop=(kt == NT - 1 and qt == 3),
                                skip_group_check=(kt == 0 and qt > 0),
                            )
                if ktg_hi < NT // KG:
                    return
                del st[okey]
                rec = ot_pool.tile([128, NQC], F32, tag="rec", name=f"rec{qc}_{b}")
                nc.vector.reciprocal(
                    rec[:], ops[:, :, H : H + 1].rearrange("p a o -> p (a o)")
                )
                ot = ot_pool.tile([128, NQC, H], F32, tag="ot", name=f"ot{qc}_{b}")
                for qt in range(4):
                    nc.vector.tensor_scalar_mul(
                        ot[:, qt, :], ops[:, qt, :H], rec[:, qt : qt + 1]
                    )
                nc.sync.dma_start(out_d.ap()[b, :, qc * 4 : (qc + 1) * 4, :], ot[:])

            # ---- emission schedule ------------------------------------------
            for q in range(4):
                emit_xbar_quarter(0, q)
            emit_xbar_full(1)

            # batch 0 projections, progressively interleaved with the C phase.
            # qc0/qc1 run in two half-key passes so exp never waits on the
            # tail of the x-transpose DMA stream.
            for u in u_qk(0, "k", 0) + u_qk(0, "q", 0):
                u()
            fills = iter(
                u_v(0, 0)
                + u_qk(0, "k", 1)
                + u_v(0, 1)
                + u_qk(0, "k", 2)
                + u_v(0, 2)
                + u_qk(0, "k", 3)
                + u_v(0, 3)
                + u_qk(0, "q", 1)      # 22 units: C0 qc0 (8) + qc1 (8) + qc2 (6)
                + u_qk(0, "q", 2)
                + u_qk(0, "q", 3)
                + u_qk(1, "k", 0)
                + u_qk(1, "q", 0)
                + u_v(1, 0)
                + u_qk(1, "k", 1)
                + u_v(1, 1)
                + u_qk(1, "k", 2)
                + u_v(1, 2)
                + u_qk(1, "k", 3)
                + u_v(1, 3)
                + u_qk(1, "q", 1)
                + u_qk(1, "q", 2)
                + u_qk(1, "q", 3)
            )
            for b in range(2):
                for qc in range(NQC):
                    c_pass(b, qc, 0, 8, fills)

    _split_excess_waits(nc)
    return nc


_prog = None


def _get_prog():
    global _prog
    if _prog is None:
        _prog = _build()
    return _prog


def _to_bf16(a):
    import ml_dtypes

    return np.asarray(a, dtype=ml_dtypes.bfloat16)


def kernel(x, mask, Wk, Wq, Wv, **_ignored):
    x = np.ascontiguousarray(x, dtype=np.float32)
    xp = np.zeros((B, T, CP), dtype=np.float32)
    xp[:, :, :C] = x
    xp = _to_bf16(xp)

    wall = np.zeros((3 * H, CP), dtype=np.float32)
    for i, W in enumerate((Wq, Wk, Wv)):
        wall[i * H : (i + 1) * H, :C] = np.asarray(W, dtype=np.float32).T
    wall = np.ascontiguousarray(_to_bf16(wall))

    nc = _get_prog()
    in_maps = [
        {"xp": xp[i * BPC : (i + 1) * BPC], "wall": wall}
        for i in range(N_CORES)
    ]
    res = run_bass_kernel_spmd(nc, in_maps, core_ids=list(range(N_CORES)))
    # out is p-major [BPC, 128, NT, H]: unpermute to [BPC, T, H]
    outs = []
    for i in range(N_CORES):
        o = res.results[i]["out"]
        outs.append(o.transpose(0, 2, 1, 3).reshape(BPC, T, H))
    return np.concatenate(outs, axis=0)


if __name__ == "__main__":
    rng = np.random.default_rng(0)
    x = rng.standard_normal((B, T, C), dtype=np.float32)
    mask = np.ones((B, T), dtype=bool)
    s = 1.0 / np.sqrt(C)
    Wk = (rng.standard_normal((C, H)) * s).astype(np.float32)
    Wq = (rng.standard_normal((C, H)) * s).astype(np.float32)
    Wv = (rng.standard_normal((C, H)) * s).astype(np.float32)
    out = kernel(x, mask=mask, Wk=Wk, Wq=Wq, Wv=Wv)
    print("out", out.shape, out.dtype, float(np.abs(out).max()))


# revision 19
# speedup vs baseline: 1.2351x; 1.0081x over previous
"""Single-head attention (B=16, T=2048, C=576, H=96) on 8 TRN2 NeuronCores.

Sharding: data-parallel over batch — 2 batches per core; weights replicated.

Per-core algorithm (bf16 compute, fp32 accumulation in PSUM):
  A. xT [C,T] loaded directly TRANSPOSED from DRAM via the DMA XBAR
     (dma_start_transpose, 2-byte dtype): no PE transposes, no PSUM->SBUF
     copies for x at all. x is host-padded C 576->640 so every c-tile is 128.
  B. qT,kT [96,T] = W.T @ xT (5 c-tile PSUM accumulation); v computed
     NATURAL [t128,96] (lhsT = xT tile, rhs = W) into vp [128,16,98] with
     columns 96,97 = 1.0 — the softmax denominator falls out of the output
     matmul for free.  mask is all-ones (spec fill=ones) and is ignored.
  C. scores TRANSPOSED: sT[k128, q512] = kT_chunk.T @ qT; exp on ScalarE
     straight out of PSUM (scale folded in), es written bf16 to SBUF.
     Output computed NATURAL via es-as-stationary: out[q128, 98] +=
     es[k,q].T @ v[k,98] accumulated over the 16 k-tiles.  Denominator is
     column 96; DVE reciprocal + scalar-mul normalize, store p-major
     (host unpermutes).  Softmax skips the max subtraction (scores ~N(0,1)).

Schedule: PE warmup matmuls hold the p-state ramp from t~0.5us; batch-0 xT
arrives as 20 quarter-slice XBAR DMAs so projections/scores start ~8us in;
batch-1 xT rides the DMA engine during batch-0 compute; batch-1 projections
fill PE gaps in batch-0's exp-bound C phase.  ScalarE (exp: 2*2048^2/128
lane-cycles @1.2GHz) is the roofline engine at ~66us busy.

This walrus build rejects >1 sync wait per instruction (and any wait on a
Drain), so after TileContext builds the module we hoist excess waits onto
injected same-engine NOPs — semantics identical since engines execute their
stream in order.
"""

import sys

if "/opt/trn_rl_repo" not in sys.path:
    sys.path.insert(0, "/opt/trn_rl_repo")

import numpy as np

import concourse.bass as bass
import concourse.tile as tile
from concourse import mybir
from concourse.bass_utils import run_bass_kernel_spmd

N_CORES = 8
B, T, C, H = 16, 2048, 576, 96
CP = 640  # C padded to a multiple of 128
BPC = B // N_CORES  # batches per core
SCALE = 1.0 / float(np.sqrt(H))

BF16 = mybir.dt.bfloat16
FP8 = mybir.dt.float8e4
F32 = mybir.dt.float32

NCT = CP // 128  # 5 c-tiles
NT = T // 128  # 16 t/k-tiles
NQC = T // 512  # 4 query chunks
KG = 2  # k-tiles per exp group
HP = H + 2  # 98: H plus denominator column (96) plus pad for even free


def _split_excess_waits(nc, max_waits=1):
    """Hoist sync waits beyond this walrus's per-instruction limit onto
    injected NOPs that run just before, on the same engine."""
    n_split = 0
    for fn in nc.m.functions:
        for blk in fn.blocks:
            new_insts = []
            changed = False
            for inst in blk.instructions:
                si = inst.sync_info
                waits = list(si.on_wait) if si is not None else []
                cap = 0 if isinstance(inst, mybir.InstDrain) else max_waits
                if len(waits) > cap:
                    excess = waits[:-cap] if cap else waits
                    keep = waits[-cap:] if cap else []
                    for i in range(0, len(excess), max_waits):
                        chunk = excess[i : i + max_waits]
                        new_insts.append(
                            mybir.InstNoOp(
                                name=f"{inst.name}-wsplit{i}",
                                engine=inst.engine,
                                ins=[],
                                outs=[],
                                sync_info=mybir.SyncInfo(on_wait=chunk, on_update=[]),
                            )
                        )
                    inst.sync_info = mybir.SyncInfo(
                        on_wait=keep, on_update=list(si.on_update)
                    )
                    changed = True
                    n_split += 1
                new_insts.append(inst)
            if changed:
                blk.instructions = new_insts
    return n_split


def _build():
    nc = bass.Bass("TRN2", target_bir_lowering=False, debug=False)

    xp_d = nc.dram_tensor("xp", [BPC, T, CP], BF16, kind="ExternalInput")
    w_d = nc.dram_tensor("wall", [3 * H, CP], BF16, kind="ExternalInput")
    out_d = nc.dram_tensor("out", [BPC, 128, NT, H], F32, kind="ExternalOutput")

    exp = mybir.ActivationFunctionType.Exp

    with tile.TileContext(nc) as tc:
        with (
            tc.tile_pool(name="const", bufs=1) as cpool,
            tc.tile_pool(name="xt", bufs=2) as xt_pool,
            tc.tile_pool(name="qk", bufs=2) as qk_pool,
            tc.tile_pool(name="vp", bufs=2) as vp_pool,
            tc.tile_pool(name="es", bufs=4) as es_pool,
            tc.tile_pool(name="ot", bufs=2) as ot_pool,
            tc.tile_pool(name="psm", bufs=2, space="PSUM") as psm,  # proj + warm
            tc.tile_pool(name="pss", bufs=2, space="PSUM") as pss,  # scores
            tc.tile_pool(name="pso", bufs=2, space="PSUM") as pso,  # out accum
        ):
            # ---- PE warmup: hold the p-state ramp until real work lands ----
            wm = cpool.tile([128, 512], BF16, name="wm")
            nc.vector.memset(wm[:], 0.0)
            for i in range(10):
                wps = psm.tile([128, 512], F32, tag="mm", name=f"warm{i}")
                nc.tensor.matmul(wps[:2, :], wm[:, 0:2], wm[:], start=True, stop=True)

            # ---- weight + xT DMAs --------------------------------------------
            # ALL DMAs ride one queue (SP) ordered transposes-first: any
            # plain<->transpose transition serializes on DMA completion
            # globally (XBAR mode switch), so weights load via the XBAR too:
            # host packs W.T [3H, CP]; transpose yields [128, ci, nm*H+h] =
            # W_nm[ci*128+p, h].
            # per-slot layout [128, nm, ci, H] so each weight-slot XBAR
            # transpose writes a CONTIGUOUS [5, 96] region (strided 3D xbar
            # outputs are not safe)
            wall = cpool.tile([128, 3, NCT, H], BF16, name="wall")
            NMI = {"q": 0, "k": 1, "v": 2}

            def emit_w_one(nm):
                i = NMI[nm]
                nc.sync.dma_start(
                    wall[:, i, :, :], w_d.ap()[i * H : (i + 1) * H, :], transpose=True
                )

            def emit_w_k():
                # k-slot rows first: the first projection (k chunk 0) gates
                # the whole pipeline start
                emit_w_one("k")

            def emit_w_qv():
                emit_w_one("q")
                emit_w_one("v")

            def w_sb(nm, ci):
                return wall[:, NMI[nm], ci, :]

            state = {0: {"xt": {}}, 1: {"xt": {}}}

            # xT arrives via the DMA XBAR with a 3D out AP: the middle dim
            # folds into the partition axis, so ONE DmaTransposeAnt moves a
            # [rows, 640] block into [128, 5, rows] (xt[p, ci, r] =
            # x[r, ci*128+p]) — one DMA issue per quarter (HWDGE holds the
            # sequencer ~650ns per DMA, so issue count matters).
            def emit_xbar_quarter(b, q):
                t = xt_pool.tile([128, NCT, 512], BF16, tag=f"xtq{q}", name=f"xtq{q}_{b}")
                state[b]["xt"][q] = t
                nc.sync.dma_start(
                    t[:], xp_d.ap()[b, q * 512 : (q + 1) * 512, :], transpose=True
                )

            def emit_xbar_full(b):
                t = xt_pool.tile([128, NCT, T], BF16, tag="xtf", name=f"xtf_{b}")
                for q in range(4):
                    state[b]["xt"][q] = t
                state[b]["xt_full"] = True
                nc.sync.dma_start(t[:], xp_d.ap()[b], transpose=True)

            def xtv(b, q, ci, lo, hi):
                """xT slice [128, hi-lo] of quarter q, c-tile ci (cols within
                the quarter)."""
                st = state[b]
                if st.get("xt_full"):
                    return st["xt"][q][:, ci, q * 512 + lo : q * 512 + hi]
                return st["xt"][q][:, ci, lo:hi]

            # ---- projection units -------------------------------------------
            # each unit is split into sub-~500ns pieces so a C-phase ktg slot
            # (exp budget 1038ns, PE base ~550ns) never overruns ScalarE
            def u_qk(b, nm, ch):
                box = {}

                def part(ci_lo, ci_hi, copy):
                    def go():
                        st = state[b]
                        key = f"t{nm}"
                        if key not in st:
                            st[key] = qk_pool.tile(
                                [H, T], BF16, tag=key, name=f"{key}_{b}"
                            )
                        if "pp" not in box:
                            box["pp"] = psm.tile(
                                [128, 512], F32, tag="mm", name=f"p{nm}{ch}_{b}"
                            )
                        pp = box["pp"]
                        for ci in range(ci_lo, ci_hi):
                            nc.tensor.matmul(
                                pp[:H, :],
                                w_sb(nm, ci),
                                xtv(b, ch, ci, 0, 512),
                                start=(ci == 0),
                                stop=(ci == NCT - 1),
                            )
                        if copy:
                            dst = st[key][:, ch * 512 : (ch + 1) * 512]
                            if b == 0 and ch == 0:
                                # ScalarE is idle until the first exp; doing
                                # this copy there shortens the critical chain
                                nc.scalar.copy(dst, pp[:H, :])
                            else:
                                nc.vector.tensor_copy(dst, pp[:H, :])

                    return go

                return [part(0, 2, False), part(2, 4, False), part(4, NCT, True)]

            def u_v(b, g):
                box = {}

                def part(j_lo, j_hi, copy):
                    def go():
                        st = state[b]
                        if "vp" not in st:
                            st["vp"] = vp_pool.tile(
                                [128, NT, HP], BF16, tag="vp", name=f"vp_{b}"
                            )
                            nc.vector.memset(st["vp"][:, :, H:HP], 1.0)
                        if "pv" not in box:
                            box["pv"] = psm.tile(
                                [128, 512], F32, tag="mm", name=f"pv{g}_{b}"
                            )
                        pv = box["pv"]
                        # 4 accumulation islands share one zero region: only
                        # the first matmul starts, only the last stops
                        for j in range(j_lo, j_hi):
                            tt = g * 4 + j
                            for ci in range(NCT):
                                nc.tensor.matmul(
                                    pv[:, j * H : (j + 1) * H],
                                    xtv(b, g, ci, j * 128, (j + 1) * 128),
                                    w_sb("v", ci),
                                    start=(ci == 0 and j == 0),
                                    stop=(ci == NCT - 1 and j == 3),
                                    skip_group_check=(ci == 0 and j > 0),
                                )
                        if copy:
                            nc.vector.tensor_copy(
                                st["vp"][:, g * 4 : (g + 1) * 4, :H],
                                pv[:, : 4 * H].rearrange("p (j h) -> p j h", h=H),
                            )

                    return go

                return [part(0, 2, False), part(2, 4, True)]

            # ---- attention C phase ------------------------------------------
            def c_pass(b, qc, ktg_lo, ktg_hi, fill):
                """ktg groups [ktg_lo, ktg_hi) of one 512-query chunk:
                (2 scores, 1 exp, 8 output-accumulate matmuls) per group; on
                the final group, normalize + store.  fill: iterator of
                emit-closures interleaved once per ktg."""
                st = state[b]
                tq, tk = st["tq"], st["tk"]
                okey = f"ops{qc}"
                if okey not in st:
                    st[okey] = pso.tile([128, NQC, HP], F32, tag="o", name=f"{okey}_{b}")
                ops = st[okey]
                for ktg in range(ktg_lo, ktg_hi):
                    sps = pss.tile([128, KG, 512], F32, tag="s", name=f"sps_{b}")
                    for j in range(KG):
                        kt = ktg * KG + j
                        nc.tensor.matmul(
                            sps[:, j, :],
                            tk[:, kt * 128 : (kt + 1) * 128],
                            tq[:, qc * 512 : (qc + 1) * 512],
                            start=True,
                            stop=True,
                        )
                    es = es_pool.tile([128, KG, 512], BF16, tag="es", name=f"es_{b}")
                    nc.scalar.activation(es[:], sps[:], exp, scale=SCALE)
                    if fill is not None:
                        u = next(fill, None)
                        if u is not None:
                            u()
                    for qt in range(4):
                        for j in range(KG):
                            kt = ktg * KG + j
                            nc.tensor.matmul(
                                ops[:, qt, :],
                                es[:, j, qt * 128 : (qt + 1) * 128],
                                st["vp"][:, kt, :],
                                start=(kt == 0 and qt == 0),
                                stop=(kt == NT - 1 and qt == 3),
                                skip_group_check=(kt == 0 and qt > 0),
                            )
                if ktg_hi < NT // KG:
                    return
                del st[okey]
                rec = ot_pool.tile([128, NQC], F32, tag="rec", name=f"rec{qc}_{b}")
                nc.vector.reciprocal(
                    rec[:], ops[:, :, H : H + 1].rearrange("p a o -> p (a o)")
                )
                ot = ot_pool.tile([128, NQC, H], F32, tag="ot", name=f"ot{qc}_{b}")
                for qt in range(4):
                    nc.vector.tensor_scalar_mul(
                        ot[:, qt, :], ops[:, qt, :H], rec[:, qt : qt + 1]
                    )
                nc.sync.dma_start(out_d.ap()[b, :, qc * 4 : (qc + 1) * 4, :], ot[:])

            # ---- emission schedule ------------------------------------------
            emit_w_k()
            emit_xbar_quarter(0, 0)
            emit_w_qv()
            for q in range(1, 4):
                emit_xbar_quarter(0, q)
            emit_xbar_full(1)

            # batch 0 projections, progressively interleaved with the C phase.
            # qc0/qc1 run in two half-key passes so exp never waits on the
            # tail of the x-transpose DMA stream.
            for u in u_qk(0, "k", 0) + u_qk(0, "q", 0):
                u()
            fills = iter(
                u_v(0, 0)
                + u_qk(0, "k", 1)
                + u_v(0, 1)
                + u_qk(0, "k", 2)
                + u_v(0, 2)
                + u_qk(0, "k", 3)
                + u_v(0, 3)
                + u_qk(0, "q", 1)      # 22 units: C0 qc0 (8) + qc1 (8) + qc2 (6)
                + u_qk(0, "q", 2)
                + u_qk(0, "q", 3)
                + u_qk(1, "k", 0)
                + u_qk(1, "q", 0)
                + u_v(1, 0)
                + u_qk(1, "k", 1)
                + u_v(1, 1)
                + u_qk(1, "k", 2)
                + u_v(1, 2)
                + u_qk(1, "k", 3)
                + u_v(1, 3)
                + u_qk(1, "q", 1)
                + u_qk(1, "q", 2)
                + u_qk(1, "q", 3)
            )
            for b in range(2):
                for qc in range(NQC):
                    c_pass(b, qc, 0, 8, fills)

    _split_excess_waits(nc)
    return nc


_prog = None


def _get_prog():
    global _prog
    if _prog is None:
        _prog = _build()
    return _prog


def _to_bf16(a):
    import ml_dtypes

    return np.asarray(a, dtype=ml_dtypes.bfloat16)


def kernel(x, mask, Wk, Wq, Wv, **_ignored):
    x = np.ascontiguousarray(x, dtype=np.float32)
    xp = np.zeros((B, T, CP), dtype=np.float32)
    xp[:, :, :C] = x
    xp = _to_bf16(xp)

    wall = np.zeros((3 * H, CP), dtype=np.float32)
    for i, W in enumerate((Wq, Wk, Wv)):
        wall[i * H : (i + 1) * H, :C] = np.asarray(W, dtype=np.float32).T
    wall = np.ascontiguousarray(_to_bf16(wall))

    nc = _get_prog()
    in_maps = [
        {"xp": xp[i * BPC : (i + 1) * BPC], "wall": wall}
        for i in range(N_CORES)
    ]
    res = run_bass_kernel_spmd(nc, in_maps, core_ids=list(range(N_CORES)))
    # out is p-major [BPC, 128, NT, H]: unpermute to [BPC, T, H]
    outs = []
    for i in range(N_CORES):
        o = res.results[i]["out"]
        outs.append(o.transpose(0, 2, 1, 3).reshape(BPC, T, H))
    return np.concatenate(outs, axis=0)


if __name__ == "__main__":
    rng = np.random.default_rng(0)
    x = rng.standard_normal((B, T, C), dtype=np.float32)
    mask = np.ones((B, T), dtype=bool)
    s = 1.0 / np.sqrt(C)
    Wk = (rng.standard_normal((C, H)) * s).astype(np.float32)
    Wq = (rng.standard_normal((C, H)) * s).astype(np.float32)
    Wv = (rng.standard_normal((C, H)) * s).astype(np.float32)
    out = kernel(x, mask=mask, Wk=Wk, Wq=Wq, Wv=Wv)
    print("out", out.shape, out.dtype, float(np.abs(out).max()))
